# revision 3
# baseline (speedup 1.0000x reference)
"""Trainium2 Bass kernel for a 2-layer LIF spiking network (DSQN forward).

Math (per batch b, feature h, timestep t; THR=1, snntorch reset='subtract'):
    cur1 = W1 @ x_t + b1                      (precomputable, no recurrence)
    mem1 <- beta1*mem1 + cur1 - H(mem1 - 1)   (reset uses PREVIOUS mem)
    spk1 = H(mem1 - 1)
    cur2 = W2 @ spk1 + b2
    mem2 <- beta2*mem2 + cur2 - H(mem2 - 1)
    spk2 = H(mem2 - 1)
    out_t = W3 @ spk2 + b3

Mapping:
  - Pure data parallel: batch 512 -> 64 per core across 8 cores.
  - Feature-major layout on chip: partitions = H (128), free = (t, b) columns.
  - Host pre-transposes state to (F, T, B64) per core so every DMA is
    contiguous; output comes back as (A, T, B64) and is untransposed on host.
  - One fused custom DVE op does a whole LIF membrane update per step:
        mem_new = mem*beta + (cur + bias) - (mem > 1)
    (beta, bias are per-partition scalars). 2 DVE ops per timestep total.
  - Spikes are extracted in bulk per chunk on ScalarE as sgn = sign(mem-1),
    using the exact +-1 reformulation: W2@spk = (0.5*W2)@sgn + 0.5*W2@1
    (exact in fp32 because scaling by 0.5 is lossless); same for W3.
  - W1/W2 matmuls in fp32 (spike threshold flips amplify weight rounding);
    W3 in bf16 (output-only, measured harmless).
"""

import numpy as np
import ml_dtypes

import concourse.bacc as bacc
import concourse.mybir as mybir
import concourse.tile as tile
from concourse.bass_utils import run_bass_kernel_spmd
from concourse.dve_spec import Spec, Src0, Src1, C0, C1, One, lower
from concourse.dve_uop import DveOpSpec
from concourse import dve_ops
from concourse.dve_ops import DveOp, OPS, _CUSTOM_DVE_ROW_BASE, _SUB_OPCODE_FOR_NAME

F32 = mybir.dt.float32
BF16 = mybir.dt.bfloat16
AF = mybir.ActivationFunctionType

N_CORES = 8
H = 128
F = 128
A = 16
B_LOC = 64          # batch per core
TC = 8              # timesteps per chunk
COLS = TC * B_LOC   # 512 columns per chunk


def _register_lif_op() -> DveOp:
    """mem_new = Src0*C0 + (Src1 + C1) - (Src0 > 1)."""
    name = "LIF_STEP_ANT"
    for o in OPS:
        if o.name == name:
            return o
    body = Src0 * C0 + (Src1 + C1) - (Src0 > One)
    spec = Spec(
        body=body,
        reference=lambda in0, in1, s0, s1: in0 * s0 + in1 + s1
        - (in0 > 1.0).astype(np.float32),
    )
    shas = {
        ver: DveOpSpec(name=name, uops=lower(spec, ver=ver), rd1_en=True).sha(ver)
        for ver in ("v3", "v4")
    }
    op = DveOp(name, spec, subdim=False, uops_sha=shas)
    OPS.append(op)
    _SUB_OPCODE_FOR_NAME[name] = _CUSTOM_DVE_ROW_BASE + len(OPS) - 1
    return op


def build_program(T: int = 1024):
    """Build the per-core SPMD Bass program. Returns compiled Bacc."""
    assert T % TC == 0
    n_chunks = T // TC
    lif = _register_lif_op()

    nc = bacc.Bacc("TRN2", target_bir_lowering=False, debug=False,
                   num_devices=N_CORES)

    x_d = nc.dram_tensor("x", (F, T, B_LOC), F32, kind="ExternalInput")
    m0_d = nc.dram_tensor("mem0", (2, H, B_LOC), F32, kind="ExternalInput")
    w1_d = nc.dram_tensor("w1t", (F, H), F32, kind="ExternalInput")
    w2_d = nc.dram_tensor("w2t", (H, H), F32, kind="ExternalInput")
    w3_d = nc.dram_tensor("w3t", (H, A), BF16, kind="ExternalInput")
    beta1_d = nc.dram_tensor("beta1", (H, 1), F32, kind="ExternalInput")
    beta2_d = nc.dram_tensor("beta2", (H, 1), F32, kind="ExternalInput")
    c1_d = nc.dram_tensor("c1", (H, 1), F32, kind="ExternalInput")
    c2_d = nc.dram_tensor("c2", (H, 1), F32, kind="ExternalInput")
    c3_d = nc.dram_tensor("c3", (A, 1), F32, kind="ExternalInput")
    out_d = nc.dram_tensor("out", (A, T, B_LOC), F32, kind="ExternalOutput")

    with tile.TileContext(nc) as tc:
        with (
            tc.tile_pool(name="consts", bufs=1) as cpool,
            tc.tile_pool(name="xin", bufs=4) as xpool,
            tc.tile_pool(name="ps1", bufs=2, space="PSUM") as ps1pool,
            tc.tile_pool(name="cur1", bufs=4) as c1pool,
            tc.tile_pool(name="mema", bufs=3) as mapool,
            tc.tile_pool(name="sgn1", bufs=3) as s1pool,
            tc.tile_pool(name="ps2", bufs=2, space="PSUM") as ps2pool,
            tc.tile_pool(name="cur2", bufs=4) as c2pool,
            tc.tile_pool(name="memb", bufs=3) as mbpool,
            tc.tile_pool(name="sgn2", bufs=3) as s2pool,
            tc.tile_pool(name="ps3", bufs=2, space="PSUM") as ps3pool,
            tc.tile_pool(name="outs", bufs=4) as opool,
        ):
            w1_s = cpool.tile([F, H], F32)
            nc.sync.dma_start(w1_s[:], w1_d.ap())
            w2_s = cpool.tile([H, H], F32)
            nc.sync.dma_start(w2_s[:], w2_d.ap())
            w3_s = cpool.tile([H, A], BF16)
            nc.sync.dma_start(w3_s[:], w3_d.ap())
            beta1_s = cpool.tile([H, 1], F32)
            nc.sync.dma_start(beta1_s[:], beta1_d.ap())
            beta2_s = cpool.tile([H, 1], F32)
            nc.sync.dma_start(beta2_s[:], beta2_d.ap())
            c1_s = cpool.tile([H, 1], F32)
            nc.sync.dma_start(c1_s[:], c1_d.ap())
            c2_s = cpool.tile([H, 1], F32)
            nc.sync.dma_start(c2_s[:], c2_d.ap())
            c3_s = cpool.tile([A, 1], F32)
            nc.sync.dma_start(c3_s[:], c3_d.ap())
            m1_s = cpool.tile([H, B_LOC], F32)
            nc.sync.dma_start(m1_s[:], m0_d.ap()[0])
            m2_s = cpool.tile([H, B_LOC], F32)
            nc.sync.dma_start(m2_s[:], m0_d.ap()[1])
            negone = cpool.tile([H, 1], F32)
            nc.gpsimd.memset(negone[:], -1.0)

            prev_a = m1_s[:]
            prev_b = m2_s[:]

            for c in range(n_chunks):
                t0 = c * TC
                x_t = xpool.tile([F, COLS], F32)
                nc.sync.dma_start(
                    x_t[:],
                    x_d.ap()[:, t0:t0 + TC, :].rearrange("p a b -> p (a b)"),
                )
                ps1 = ps1pool.tile([H, COLS], F32)
                nc.tensor.matmul(ps1[:], w1_s[:], x_t[:], start=True, stop=True)
                cur1 = c1pool.tile([H, COLS], F32)
                nc.scalar.activation(cur1[:], ps1[:], AF.Copy)

                ma = mapool.tile([H, COLS], F32)
                for i in range(TC):
                    src0 = prev_a if i == 0 else ma[:, (i - 1) * B_LOC:i * B_LOC]
                    nc.vector._custom_dve(
                        lif,
                        out=ma[:, i * B_LOC:(i + 1) * B_LOC],
                        in0=src0,
                        in1=cur1[:, i * B_LOC:(i + 1) * B_LOC],
                        s0=beta1_s[:, 0:1],
                        s1=c1_s[:, 0:1],
                    )
                prev_a = ma[:, (TC - 1) * B_LOC:TC * B_LOC]

                sg1 = s1pool.tile([H, COLS], F32)
                nc.scalar.activation(sg1[:], ma[:], AF.Sign, bias=negone[:, 0:1])

                ps2 = ps2pool.tile([H, COLS], F32)
                nc.tensor.matmul(ps2[:], w2_s[:], sg1[:], start=True, stop=True)
                cur2 = c2pool.tile([H, COLS], F32)
                nc.scalar.activation(cur2[:], ps2[:], AF.Copy)

                mb = mbpool.tile([H, COLS], F32)
                for i in range(TC):
                    src0 = prev_b if i == 0 else mb[:, (i - 1) * B_LOC:i * B_LOC]
                    nc.vector._custom_dve(
                        lif,
                        out=mb[:, i * B_LOC:(i + 1) * B_LOC],
                        in0=src0,
                        in1=cur2[:, i * B_LOC:(i + 1) * B_LOC],
                        s0=beta2_s[:, 0:1],
                        s1=c2_s[:, 0:1],
                    )
                prev_b = mb[:, (TC - 1) * B_LOC:TC * B_LOC]

                sg2 = s2pool.tile([H, COLS], BF16)
                nc.scalar.activation(sg2[:], mb[:], AF.Sign, bias=negone[:, 0:1])

                ps3 = ps3pool.tile([A, COLS], F32)
                nc.tensor.matmul(ps3[:], w3_s[:], sg2[:], start=True, stop=True)
                out_t = opool.tile([A, COLS], F32)
                nc.scalar.activation(out_t[:], ps3[:], AF.Identity,
                                     bias=c3_s[:, 0:1])
                nc.sync.dma_start(
                    out_d.ap()[:, t0:t0 + TC, :].rearrange("p a b -> p (a b)"),
                    out_t[:],
                )

    nc.compile()
    return nc


def make_in_maps(state_batch, hidden_states, W1, b1, beta1, W2, b2, beta2,
                 W3, b3, T=None):
    """Host-side prep: shard/transpose per core, fold constants (exact)."""
    x = np.asarray(state_batch, np.float32)
    hs = np.asarray(hidden_states, np.float32)
    B = x.shape[0]
    if T is None:
        T = x.shape[1]
    W1 = np.asarray(W1, np.float32)
    W2 = np.asarray(W2, np.float32)
    W3 = np.asarray(W3, np.float32)

    w1t = np.ascontiguousarray(W1.T)
    w2t = np.ascontiguousarray((0.5 * W2).T)
    w3t = np.ascontiguousarray((0.5 * W3).T).astype(ml_dtypes.bfloat16)
    be1 = np.clip(np.asarray(beta1, np.float32), 0.0, 1.0).reshape(H, 1)
    be2 = np.clip(np.asarray(beta2, np.float32), 0.0, 1.0).reshape(H, 1)
    c1 = np.asarray(b1, np.float32).reshape(H, 1)
    c2 = (np.asarray(b2, np.float64)
          + 0.5 * np.asarray(W2, np.float64).sum(1)).astype(np.float32).reshape(H, 1)
    c3 = (np.asarray(b3, np.float64)
          + 0.5 * np.asarray(W3, np.float64).sum(1)).astype(np.float32).reshape(A, 1)

    in_maps = []
    for c in range(N_CORES):
        bs = slice(c * B_LOC, (c + 1) * B_LOC)
        xc = np.ascontiguousarray(x[bs, :T].transpose(2, 1, 0))      # (F,T,B)
        m0 = np.ascontiguousarray(hs[bs, 0].transpose(1, 2, 0))      # (2,H,B)
        in_maps.append({
            "x": xc, "mem0": m0, "w1t": w1t, "w2t": w2t, "w3t": w3t,
            "beta1": be1, "beta2": be2, "c1": c1, "c2": c2, "c3": c3,
        })
    return in_maps


def assemble_output(results, B, T):
    out = np.empty((B, T, A), np.float32)
    for c in range(len(results)):
        bs = slice(c * B_LOC, (c + 1) * B_LOC)
        out[bs] = results[c]["out"].transpose(2, 1, 0)               # (B,T,A)
    return out


_NC_CACHE = {}


def kernel(**inputs) -> np.ndarray:
    x = np.asarray(inputs["state_batch"], np.float32)
    B, T, _ = x.shape
    if T not in _NC_CACHE:
        _NC_CACHE[T] = build_program(T)
    nc = _NC_CACHE[T]
    in_maps = make_in_maps(**inputs, T=T)
    res = run_bass_kernel_spmd(nc, in_maps, core_ids=list(range(N_CORES)),
                               trace=False)
    return assemble_output(res.results, B, T)


# revision 4
# speedup vs baseline: 2.6629x; 2.6629x over previous
"""Trainium2 Bass kernel for a 2-layer LIF spiking network (DSQN forward).

Math (per batch b, feature h, timestep t; THR=1, snntorch reset='subtract'):
    cur1 = W1 @ x_t + b1                      (precomputable, no recurrence)
    mem1 <- beta1*mem1 + cur1 - H(mem1 - 1)   (reset uses PREVIOUS mem)
    spk1 = H(mem1 - 1)
    cur2 = W2 @ spk1 + b2
    mem2 <- beta2*mem2 + cur2 - H(mem2 - 1)
    spk2 = H(mem2 - 1)
    out_t = W3 @ spk2 + b3

Mapping:
  - Pure data parallel: batch 512 -> 64 per core across 8 cores.
  - Feature-major layout on chip: partitions = H (128), free = (t, b) columns.
  - Host pre-transposes state to (F, T, B64) per core so every DMA is
    contiguous; output comes back as (A, T, B64) and is untransposed on host.
  - One fused custom DVE op does a whole LIF membrane update per step:
        mem_new = mem*beta + (cur + bias) - (mem > 1)
    (beta, bias are per-partition scalars). 2 DVE ops per timestep total.
  - Spikes are extracted in bulk per chunk on ScalarE as sgn = sign(mem-1),
    using the exact +-1 reformulation: W2@spk = (0.5*W2)@sgn + 0.5*W2@1
    (exact in fp32 because scaling by 0.5 is lossless); same for W3.
  - W1/W2 matmuls in fp32 (spike threshold flips amplify weight rounding);
    W3 in bf16 (output-only, measured harmless).
"""

import numpy as np
import ml_dtypes

import concourse.bacc as bacc
import concourse.mybir as mybir
import concourse.tile as tile
from concourse.bass_utils import run_bass_kernel_spmd
from concourse.dve_spec import Spec, Src0, Src1, C0, C1, One, lower
from concourse.dve_uop import DveOpSpec
from concourse import dve_ops
from concourse.dve_ops import DveOp, OPS, _CUSTOM_DVE_ROW_BASE, _SUB_OPCODE_FOR_NAME

F32 = mybir.dt.float32
BF16 = mybir.dt.bfloat16
AF = mybir.ActivationFunctionType

N_CORES = 8
H = 128
F = 128
A = 16
B_LOC = 64          # batch per core
TC = 8              # timesteps per chunk
COLS = TC * B_LOC   # 512 columns per chunk


def _register_lif_op() -> DveOp:
    """mem_new = Src0*C0 + (Src1 + C1) - (Src0 > 1)."""
    name = "LIF_STEP_ANT"
    for o in OPS:
        if o.name == name:
            return o
    body = Src0 * C0 + (Src1 + C1) - (Src0 > One)
    spec = Spec(
        body=body,
        reference=lambda in0, in1, s0, s1: in0 * s0 + in1 + s1
        - (in0 > 1.0).astype(np.float32),
    )
    shas = {
        ver: DveOpSpec(name=name, uops=lower(spec, ver=ver), rd1_en=True).sha(ver)
        for ver in ("v3", "v4")
    }
    op = DveOp(name, spec, subdim=False, uops_sha=shas)
    OPS.append(op)
    _SUB_OPCODE_FOR_NAME[name] = _CUSTOM_DVE_ROW_BASE + len(OPS) - 1
    return op


def build_program(T: int = 1024):
    """Build the per-core SPMD Bass program. Returns compiled Bacc."""
    assert T % TC == 0
    n_chunks = T // TC
    lif = _register_lif_op()

    nc = bacc.Bacc("TRN2", target_bir_lowering=False, debug=False,
                   num_devices=N_CORES)

    x_d = nc.dram_tensor("x", (F, T, B_LOC), F32, kind="ExternalInput")
    m0_d = nc.dram_tensor("mem0", (2, H, B_LOC), F32, kind="ExternalInput")
    w1_d = nc.dram_tensor("w1t", (F, H), F32, kind="ExternalInput")
    w2_d = nc.dram_tensor("w2t", (H, H), F32, kind="ExternalInput")
    w3_d = nc.dram_tensor("w3t", (H, A), BF16, kind="ExternalInput")
    beta1_d = nc.dram_tensor("beta1", (H, 1), F32, kind="ExternalInput")
    beta2_d = nc.dram_tensor("beta2", (H, 1), F32, kind="ExternalInput")
    c1_d = nc.dram_tensor("c1", (H, 1), F32, kind="ExternalInput")
    c2_d = nc.dram_tensor("c2", (H, 1), F32, kind="ExternalInput")
    c3_d = nc.dram_tensor("c3", (A, 1), F32, kind="ExternalInput")
    out_d = nc.dram_tensor("out", (A, T, B_LOC), F32, kind="ExternalOutput")

    with tile.TileContext(nc) as tc:
        with (
            tc.tile_pool(name="consts", bufs=1) as cpool,
            tc.tile_pool(name="xin", bufs=4) as xpool,
            tc.tile_pool(name="ps1", bufs=2, space="PSUM") as ps1pool,
            tc.tile_pool(name="cur1", bufs=4) as c1pool,
            tc.tile_pool(name="mema", bufs=3) as mapool,
            tc.tile_pool(name="sgn1", bufs=3) as s1pool,
            tc.tile_pool(name="ps2", bufs=2, space="PSUM") as ps2pool,
            tc.tile_pool(name="cur2", bufs=4) as c2pool,
            tc.tile_pool(name="memb", bufs=3) as mbpool,
            tc.tile_pool(name="sgn2", bufs=3) as s2pool,
            tc.tile_pool(name="ps3", bufs=2, space="PSUM") as ps3pool,
            tc.tile_pool(name="outs", bufs=4) as opool,
        ):
            w1_s = cpool.tile([F, H], F32)
            nc.sync.dma_start(w1_s[:], w1_d.ap())
            w2_s = cpool.tile([H, H], F32)
            nc.sync.dma_start(w2_s[:], w2_d.ap())
            w3_s = cpool.tile([H, A], BF16)
            nc.sync.dma_start(w3_s[:], w3_d.ap())
            beta1_s = cpool.tile([H, 1], F32)
            nc.sync.dma_start(beta1_s[:], beta1_d.ap())
            beta2_s = cpool.tile([H, 1], F32)
            nc.sync.dma_start(beta2_s[:], beta2_d.ap())
            c1_s = cpool.tile([H, 1], F32)
            nc.sync.dma_start(c1_s[:], c1_d.ap())
            c2_s = cpool.tile([H, 1], F32)
            nc.sync.dma_start(c2_s[:], c2_d.ap())
            c3_s = cpool.tile([A, 1], F32)
            nc.sync.dma_start(c3_s[:], c3_d.ap())
            m1_s = cpool.tile([H, B_LOC], F32)
            nc.sync.dma_start(m1_s[:], m0_d.ap()[0])
            m2_s = cpool.tile([H, B_LOC], F32)
            nc.sync.dma_start(m2_s[:], m0_d.ap()[1])
            negone = cpool.tile([H, 1], F32)
            nc.gpsimd.memset(negone[:], -1.0)

            prev_a = m1_s[:]
            prev_b = m2_s[:]

            # Layer-2 (B) work runs LAG chunks behind layer-1 (A), and the
            # two LIF chains are interleaved op-by-op on the DVE so
            # consecutive DVE instructions never form a RAW chain (hides
            # the per-op pipeline drain).
            LAG = 2
            cur2_q = {}   # chunk -> cur2 sbuf tile
            mb_q = {}     # chunk -> layer2 mem tile

            for c in range(n_chunks + LAG):
                ca = c            # layer-1 chunk being produced
                cb = c - LAG      # layer-2 chunk being produced
                if ca < n_chunks:
                    t0 = ca * TC
                    x_t = xpool.tile([F, COLS], F32)
                    nc.sync.dma_start(
                        x_t[:],
                        x_d.ap()[:, t0:t0 + TC, :].rearrange("p a b -> p (a b)"),
                    )
                    ps1 = ps1pool.tile([H, COLS], F32)
                    nc.tensor.matmul(ps1[:], w1_s[:], x_t[:],
                                     start=True, stop=True)
                    cur1 = c1pool.tile([H, COLS], F32)
                    nc.scalar.activation(cur1[:], ps1[:], AF.Copy)
                    ma = mapool.tile([H, COLS], F32)
                if cb >= 0:
                    mb = mbpool.tile([H, COLS], F32)
                    mb_q[cb] = mb
                    cur2_b = cur2_q.pop(cb)

                for i in range(TC):
                    if ca < n_chunks:
                        src0 = prev_a if i == 0 else ma[:, (i - 1) * B_LOC:i * B_LOC]
                        nc.vector._custom_dve(
                            lif,
                            out=ma[:, i * B_LOC:(i + 1) * B_LOC],
                            in0=src0,
                            in1=cur1[:, i * B_LOC:(i + 1) * B_LOC],
                            s0=beta1_s[:, 0:1],
                            s1=c1_s[:, 0:1],
                        )
                    if cb >= 0:
                        src0 = prev_b if i == 0 else mb[:, (i - 1) * B_LOC:i * B_LOC]
                        nc.vector._custom_dve(
                            lif,
                            out=mb[:, i * B_LOC:(i + 1) * B_LOC],
                            in0=src0,
                            in1=cur2_b[:, i * B_LOC:(i + 1) * B_LOC],
                            s0=beta2_s[:, 0:1],
                            s1=c2_s[:, 0:1],
                        )
                if ca < n_chunks:
                    prev_a = ma[:, (TC - 1) * B_LOC:TC * B_LOC]
                if cb >= 0:
                    prev_b = mb[:, (TC - 1) * B_LOC:TC * B_LOC]

                if ca < n_chunks:
                    sg1 = s1pool.tile([H, COLS], F32)
                    nc.scalar.activation(sg1[:], ma[:], AF.Sign,
                                         bias=negone[:, 0:1])
                    ps2 = ps2pool.tile([H, COLS], F32)
                    nc.tensor.matmul(ps2[:], w2_s[:], sg1[:],
                                     start=True, stop=True)
                    cur2 = c2pool.tile([H, COLS], F32)
                    nc.scalar.activation(cur2[:], ps2[:], AF.Copy)
                    cur2_q[ca] = cur2

                if cb >= 0:
                    mb = mb_q.pop(cb)
                    sg2 = s2pool.tile([H, COLS], BF16)
                    nc.scalar.activation(sg2[:], mb[:], AF.Sign,
                                         bias=negone[:, 0:1])
                    ps3 = ps3pool.tile([A, COLS], F32)
                    nc.tensor.matmul(ps3[:], w3_s[:], sg2[:],
                                     start=True, stop=True)
                    out_t = opool.tile([A, COLS], F32)
                    nc.scalar.activation(out_t[:], ps3[:], AF.Identity,
                                         bias=c3_s[:, 0:1])
                    tb0 = cb * TC
                    nc.sync.dma_start(
                        out_d.ap()[:, tb0:tb0 + TC, :].rearrange(
                            "p a b -> p (a b)"),
                        out_t[:],
                    )

    nc.compile()
    return nc


def make_in_maps(state_batch, hidden_states, W1, b1, beta1, W2, b2, beta2,
                 W3, b3, T=None):
    """Host-side prep: shard/transpose per core, fold constants (exact)."""
    x = np.asarray(state_batch, np.float32)
    hs = np.asarray(hidden_states, np.float32)
    B = x.shape[0]
    if T is None:
        T = x.shape[1]
    W1 = np.asarray(W1, np.float32)
    W2 = np.asarray(W2, np.float32)
    W3 = np.asarray(W3, np.float32)

    w1t = np.ascontiguousarray(W1.T)
    w2t = np.ascontiguousarray((0.5 * W2).T)
    w3t = np.ascontiguousarray((0.5 * W3).T).astype(ml_dtypes.bfloat16)
    be1 = np.clip(np.asarray(beta1, np.float32), 0.0, 1.0).reshape(H, 1)
    be2 = np.clip(np.asarray(beta2, np.float32), 0.0, 1.0).reshape(H, 1)
    c1 = np.asarray(b1, np.float32).reshape(H, 1)
    c2 = (np.asarray(b2, np.float64)
          + 0.5 * np.asarray(W2, np.float64).sum(1)).astype(np.float32).reshape(H, 1)
    c3 = (np.asarray(b3, np.float64)
          + 0.5 * np.asarray(W3, np.float64).sum(1)).astype(np.float32).reshape(A, 1)

    in_maps = []
    for c in range(N_CORES):
        bs = slice(c * B_LOC, (c + 1) * B_LOC)
        xc = np.ascontiguousarray(x[bs, :T].transpose(2, 1, 0))      # (F,T,B)
        m0 = np.ascontiguousarray(hs[bs, 0].transpose(1, 2, 0))      # (2,H,B)
        in_maps.append({
            "x": xc, "mem0": m0, "w1t": w1t, "w2t": w2t, "w3t": w3t,
            "beta1": be1, "beta2": be2, "c1": c1, "c2": c2, "c3": c3,
        })
    return in_maps


def assemble_output(results, B, T):
    out = np.empty((B, T, A), np.float32)
    for c in range(len(results)):
        bs = slice(c * B_LOC, (c + 1) * B_LOC)
        out[bs] = results[c]["out"].transpose(2, 1, 0)               # (B,T,A)
    return out


_NC_CACHE = {}


def kernel(**inputs) -> np.ndarray:
    x = np.asarray(inputs["state_batch"], np.float32)
    B, T, _ = x.shape
    if T not in _NC_CACHE:
        _NC_CACHE[T] = build_program(T)
    nc = _NC_CACHE[T]
    in_maps = make_in_maps(**inputs, T=T)
    res = run_bass_kernel_spmd(nc, in_maps, core_ids=list(range(N_CORES)),
                               trace=False)
    return assemble_output(res.results, B, T)
